# revision 29
# baseline (speedup 1.0000x reference)
"""Multi-head attention on 8 Trainium2 NeuronCores.

Problem: B=2, T=2048, D=1024, H=16 heads (dh=64), int 0/1 attention mask.

Sharding (hardcoded): core c -> batch b = c//4, head block hb = c%4
(4 heads = 256 cols per core). Wq/Wk/Wv column-sharded, Wo row-sharded;
each core returns a partial [T, D] output, host sums the 4 partials per
batch and adds bo.

Per-core kernel (all matmul inputs fp16, fp32 accumulation):
  phase 1: K^T/Q^T projections (scale folded into Wq/bq on the host, bias
           added during the PSUM->SBUF copy on DVE via tensor_scalar add).
           Only (K,Q) x (qc0,qc1) + V tiles 0-3 run up front; the
           remaining chunks drain one-per-step into phase-2 PE slack.
  phase 2: steps ordered (T-half, head, kt).  Per step:
           S^T[k, q-half] = K_h[kt] Q_h^T   ([128,1024] PSUM)
           E = exp(S^T) (ACT)  ->  E *= mask^T tile (DVE, fp16 2x)
           per q-tile: U[q,0:64] += E_tile^T V_h[kt]  (E stationary, n=64)
                       den[q]    += E_tile^T ones     (n=1)
           at kt=15: DVE reciprocal(den) + per-q-tile tensor_scalar_mul
           -> norm tile [q, pair, dh]; per head-pair DMA-xbar transpose
           [128q,128c] -> hc[c, q].
  phase 3: O_partial = hc^T.T Wo.  t-tiles 0-7 interleave into the second
           T-half's PE slack; t-tiles 8-15 run in the tail.  DMA out fp16
           (summed in f32 on the host).

No max-subtraction is needed: scores are O(1) and the masked-
multiplicative form E = exp(S) * m / sum(exp(S) * m) is exact.
"""
import contextlib
import os
import sys
import time

os.environ.setdefault("NEURON_RT_RESET_CORES", "1")

if "/opt/trn_rl_repo" not in sys.path:
    sys.path.insert(0, "/opt/trn_rl_repo")

import numpy as np

import concourse.bass as bass  # noqa: F401  (import keeps bass registered)
from concourse import bacc
import concourse.mybir as mybir
import concourse.tile as tile
from concourse.bass_utils import run_bass_kernel_spmd

f32 = mybir.dt.float32
f16 = mybir.dt.float16
AF = mybir.ActivationFunctionType

B, T, D, H = 2, 2048, 1024, 16
DH = 64                 # head dim
NHC = 4                 # heads per core
C = NHC * DH            # 256 columns per core
KD = D // 128           # 8 contraction tiles over D
KT = T // 128           # 16 k-tiles over T
QC = T // 512           # 4 q chunks of 512
QT = T // 128           # 16 q tiles of 128
NCORES = 8
SCALE = DH ** -0.5      # 0.125

_CACHE = {}


def _build(repeat=1):
    nc = bacc.Bacc()
    xt = nc.declare_dram_parameter("xt", [D, T], f16, isOutput=False)
    wq = nc.declare_dram_parameter("wq", [D, C], f16, isOutput=False)
    wk = nc.declare_dram_parameter("wk", [D, C], f16, isOutput=False)
    wv = nc.declare_dram_parameter("wv", [D, C], f16, isOutput=False)
    wo = nc.declare_dram_parameter("wo", [C, D], f16, isOutput=False)
    maskt = nc.declare_dram_parameter("maskt", [T, T], f16, isOutput=False)
    bqs = nc.declare_dram_parameter("bqs", [C], f32, isOutput=False)
    bks = nc.declare_dram_parameter("bks", [C], f32, isOutput=False)
    bvr = nc.declare_dram_parameter("bvr", [1, C], f16, isOutput=False)
    ident = nc.declare_dram_parameter("ident", [128, 128], f16, isOutput=False)
    out = nc.declare_dram_parameter("out", [T, D], f16, isOutput=True)

    with tile.TileContext(nc) as tc:
        loop_ctx = tc.For_i(0, repeat, 1) if repeat > 1 else contextlib.nullcontext()
        with (
            loop_ctx,
            tc.tile_pool(name="persist", bufs=1) as pp,
            tc.tile_pool(name="e", bufs=12) as ep,
            tc.tile_pool(name="norm", bufs=2) as npool,
            tc.tile_pool(name="osb", bufs=4) as op_,
            tc.tile_pool(name="small", bufs=4) as sp,
        ):
            xt_sb = pp.tile([128, KD, T], f16)
            wq_sb = pp.tile([128, KD, C], f16)
            wk_sb = pp.tile([128, KD, C], f16)
            wv_sb = pp.tile([128, KD, C], f16)
            wo_sb = pp.tile([128, C // 128, D], f16)
            mk_sb = pp.tile([128, KT, T], f16)
            qt_sb = pp.tile([128, C // 128, T], f16)
            kt_sb = pp.tile([128, C // 128, T], f16)
            v_sb = pp.tile([128, KT, NHC, DH + 1], f16)
            hc_sb = pp.tile([128, C // 128, T], f16)
            bq_sb = pp.tile([128, C // 128], f32)
            bk_sb = pp.tile([128, C // 128], f32)
            bv_sb = pp.tile([1, C], f16)
            ones128 = pp.tile([1, 128], f16)
            id_sb = pp.tile([128, 128], f16)

            # ---- input DMAs ----
            xt_r = xt.rearrange("(kd p) t -> p kd t", p=128)
            wq_r = wq.rearrange("(kd p) c -> p kd c", p=128)
            wk_r = wk.rearrange("(kd p) c -> p kd c", p=128)
            wv_r = wv.rearrange("(kd p) c -> p kd c", p=128)
            # DMA transfers serialize on one device (~bytes/360GB/s);
            # SWDGE (gpsimd) DMAs issue at t=0 and would interleave with the
            # startup-critical stream, so EVERYTHING goes on the one sync
            # HWDGE queue in exact deadline order.
            mk_r = maskt.rearrange("(kt p) t -> p kt t", p=128)
            nc.sync.dma_start(out=wk_sb[:, 0:2, :], in_=wk_r[:, 0:2, :])
            nc.sync.dma_start(out=wk_sb[:, 2:KD, :], in_=wk_r[:, 2:KD, :])
            for kd2 in range(0, KD, 2):
                nc.sync.dma_start(
                    out=xt_sb[:, kd2 : kd2 + 2, 0:512],
                    in_=xt_r[:, kd2 : kd2 + 2, 0:512],
                )
            nc.sync.dma_start(out=bk_sb, in_=bks.rearrange("(m p) -> p m", p=128))
            nc.sync.dma_start(out=bq_sb, in_=bqs.rearrange("(m p) -> p m", p=128))
            nc.sync.dma_start(out=wv_sb, in_=wv_r)
            nc.sync.dma_start(out=wq_sb, in_=wq_r)
            for kd4 in range(0, KD, 4):
                nc.sync.dma_start(
                    out=xt_sb[:, kd4 : kd4 + 4, 512:1024],
                    in_=xt_r[:, kd4 : kd4 + 4, 512:1024],
                )
            nc.sync.dma_start(out=bv_sb, in_=bvr[:, :])
            for kt in range(0, 6):
                nc.sync.dma_start(out=mk_sb[:, kt, :], in_=mk_r[:, kt, :])
            nc.sync.dma_start(
                out=xt_sb[:, :, 1024:1536], in_=xt_r[:, :, 1024:1536]
            )
            for kt in range(6, 9):
                nc.sync.dma_start(out=mk_sb[:, kt, :], in_=mk_r[:, kt, :])
            nc.sync.dma_start(
                out=xt_sb[:, :, 1536:2048], in_=xt_r[:, :, 1536:2048]
            )
            for kt in range(9, KT):
                nc.sync.dma_start(out=mk_sb[:, kt, :], in_=mk_r[:, kt, :])
            nc.sync.dma_start(out=wo_sb, in_=wo.rearrange("(m p) d -> p m d", p=128))
            nc.sync.dma_start(out=id_sb, in_=ident[:, :])
            nc.vector.memset(ones128, 1.0)
            nc.vector.memset(v_sb[:, :, :, DH : DH + 1], 1.0)

            # ---- projection chunk helpers ----
            def qk_sub(pool, w_sb, b_sb, dst, qc, m, half, state={}):
                # half 0 emits kd 0-3 (allocates the PSUM tile), half 1 emits
                # kd 4-7 + the bias/convert copy -- lets a 1.7us chunk drain
                # across two fill slots
                if half == 0:
                    state[(qc, m)] = pool.tile([128, 512], f32, tag="p", name="pt")
                pt = state[(qc, m)]
                for kd in range(half * 4, half * 4 + 4):
                    nc.tensor.matmul(
                        pt,
                        w_sb[:, kd, m * 128 : (m + 1) * 128],
                        xt_sb[:, kd, qc * 512 : (qc + 1) * 512],
                        start=(kd == 0),
                        stop=(kd == KD - 1),
                    )
                if half == 1:
                    state.pop((qc, m))
                    # bias add + f16 convert on DVE (bias/scale pre-folded on
                    # host); broadcast add avoids TensorScalarPtr SEQ cost
                    nc.vector.tensor_tensor(
                        dst[:, m, qc * 512 : (qc + 1) * 512],
                        pt,
                        b_sb[:, m : m + 1].to_broadcast((128, 512)),
                        mybir.AluOpType.add,
                    )

            def qk_chunk(pool, w_sb, b_sb, dst, qc, m):
                qk_sub(pool, w_sb, b_sb, dst, qc, m, 0)
                qk_sub(pool, w_sb, b_sb, dst, qc, m, 1)

            def v_sub(pool, t, half, state={}):
                if half == 0:
                    state[t] = pool.tile([128, C], f32, tag="v", name="pv")
                pv = state[t]
                for kd in range(half * 4, half * 4 + 4):
                    nc.tensor.matmul(
                        pv,
                        xt_sb[:, kd, t * 128 : (t + 1) * 128],
                        wv_sb[:, kd, :],
                        start=(kd == 0),
                        stop=False,
                    )
                if half == 1:
                    state.pop(t)
                    nc.tensor.matmul(pv, ones128, bv_sb, start=False, stop=True)
                    nc.vector.tensor_copy(
                        v_sb[:, t, :, 0:DH],
                        pv.rearrange("p (h x) -> p h x", x=DH),
                    )

            def v_proj(pool, t):
                v_sub(pool, t, 0)
                v_sub(pool, t, 1)

            # ---- phase 1 (startup): the minimum for step (th0,h0,kt0) ----
            # m=1 chunks (head pair 2/3) are only needed from step 32, V and
            # the remaining K/Q chunks drain into phase-2 PE slack (a fill
            # feeding step i MUST be emitted at a fill slot <= i: the tile
            # scheduler keeps per-engine emission order).
            with tc.tile_pool(name="ps1", bufs=4, space="PSUM") as ps1:
                # PE p-state warmup on zeros during the DMA lead (the tensor
                # engine ramps 0.65->2.4GHz over ~3us of continuous work),
                # and pre-load the Exp activation table off the critical path
                junk = sp.tile([128, 512], f16, tag="junk", name="junk")
                escr = sp.tile([1, 128], f16, tag="escr", name="escr")
                nc.vector.memset(junk, 0.0)
                nc.vector.memset(escr, 0.0)
                nc.scalar.activation(escr, escr, AF.Exp)
                wut = ps1.tile([128, 512], f32, tag="p", name="wut")
                for w in range(9):
                    nc.tensor.matmul(
                        wut, junk[:, 0:128], junk, start=(w == 0), stop=(w == 8)
                    )
                qk_chunk(ps1, wk_sb, bk_sb, kt_sb, 0, 0)
                qk_chunk(ps1, wq_sb, bq_sb, qt_sb, 0, 0)
                qk_chunk(ps1, wq_sb, bq_sb, qt_sb, 1, 0)

            # ---- phase 2 ----
            steps = [
                (th, h, kt)
                for th in range(2)
                for h in range(NHC)
                for kt in range(KT)
            ]
            nsteps = len(steps)

            with (
                tc.tile_pool(name="ps_s", bufs=2, space="PSUM") as pss,
                tc.tile_pool(name="ps_u", bufs=1, space="PSUM") as psu,
            ):
                def s_matmuls(th, h, kt):
                    m, p0 = h // 2, (h % 2) * 64
                    with tc.high_priority(offset=24):
                        st = pss.tile([128, 1024], f32, tag="s")
                        for sub in range(2):
                            q0 = th * 1024 + sub * 512
                            nc.tensor.matmul(
                                st[:, sub * 512 : (sub + 1) * 512],
                                kt_sb[p0 : p0 + 64, m, kt * 128 : (kt + 1) * 128],
                                qt_sb[p0 : p0 + 64, m, q0 : q0 + 512],
                                start=True,
                                stop=True,
                            )
                    return st

                norm_tiles = {}
                u = None

                p3_state = {}

                def phase3_half(pool, t, n, copy_engine=None):
                    if n == 0:
                        p3_state[t] = op_.tile([128, 1024], f16, name="ot")
                    ot = p3_state[t]
                    po = pool.tile([128, 512], f32, tag="o")
                    for m in range(C // 128):
                        nc.tensor.matmul(
                            po,
                            hc_sb[:, m, t * 128 : (t + 1) * 128],
                            wo_sb[:, m, n * 512 : (n + 1) * 512],
                            start=(m == 0),
                            stop=(m == C // 128 - 1),
                        )
                    eng = copy_engine or nc.vector
                    if eng is nc.scalar:
                        nc.scalar.activation(
                            ot[:, n * 512 : (n + 1) * 512], po, AF.Identity
                        )
                    else:
                        eng.tensor_copy(ot[:, n * 512 : (n + 1) * 512], po)
                    if n == 1:
                        p3_state.pop(t)
                        nc.sync.dma_start(
                            out=out[t * 128 : (t + 1) * 128, :], in_=ot
                        )

                def transpose_quad(pst, m, th, lo):
                    # PE xbar-free transpose of 4 [128q,128c] norm tiles into
                    # hc layout; Pool copies them out of PSUM.
                    nt = norm_tiles[m]
                    tp = pst.tile([128, 8, 128], f16, tag="t", name="tp")
                    for j in range(4):
                        qt = th * 8 + lo + j
                        nc.tensor.matmul(
                            tp[:, lo + j, :],
                            nt[:, qt, :, :],
                            id_sb,
                            is_transpose=True,
                        )
                        nc.vector.tensor_copy(
                            hc_sb[:, m, qt * 128 : (qt + 1) * 128],
                            tp[:, lo + j, :],
                        )

                def do_step(i, fill_fn):
                    nonlocal u
                    th, h, kt = steps[i]
                    m = h // 2
                    if kt == 0:
                        # [q, 8*64 U | pad | 8 denom]  (2 PSUM banks)
                        u = psu.tile([128, 528], f32, tag="u")
                    st_cur = do_step.st_next
                    if i + 1 < nsteps:
                        do_step.st_next = s_matmuls(*steps[i + 1])
                    if fill_fn is not None:
                        fill_fn()
                    e = ep.tile([128, 1024], f16)
                    nc.scalar.activation(e, st_cur, AF.Exp)
                    nc.vector.tensor_mul(
                        e, e, mk_sb[:, kt, th * 1024 : (th + 1) * 1024]
                    )
                    # PSUM zero-regions are 2KB banks: only the first chain
                    # in each bank may set start=True (it pending-zeroes the
                    # whole bank); sibling chains at kt=0 then overwrite via
                    # the pending-zero flags.
                    for qtl in range(8):
                        nc.tensor.matmul(
                            u[:, qtl * DH : (qtl + 1) * DH],
                            e[:, qtl * 128 : (qtl + 1) * 128],
                            v_sb[:, kt, h, 0:DH],
                            start=(kt == 0 and qtl == 0),
                            stop=(kt == KT - 1),
                            skip_group_check=True,
                        )
                        nc.tensor.matmul(
                            u[:, 512 + qtl : 513 + qtl],
                            e[:, qtl * 128 : (qtl + 1) * 128],
                            v_sb[:, kt, h, DH : DH + 1],
                            start=(kt == 0 and qtl == 0),
                            stop=(kt == KT - 1),
                            skip_group_check=True,
                        )
                    if kt == KT - 1:
                        with tc.high_priority(offset=40):
                            if m not in norm_tiles:
                                norm_tiles[m] = npool.tile(
                                    [128, QT, 2, DH], f16, tag="n", name=f"nt{m}"
                                )
                            nt = norm_tiles[m]
                            rec = sp.tile([128, 8, 1], f32, tag="r")
                            nc.vector.reciprocal(
                                rec[:, :, 0], u[:, 512:520]
                            )
                            # single broadcast multiply (recip varies along
                            # the qt free dim -> 0-stride broadcast over dh)
                            nc.vector.tensor_tensor(
                                nt[:, th * 8 : (th + 1) * 8, h % 2, :],
                                u[:, 0:512].rearrange("p (a b) -> p a b", b=DH),
                                rec.to_broadcast((128, 8, DH)),
                                mybir.AluOpType.mult,
                            )

                do_step.st_next = s_matmuls(*steps[0])

                # --- T-half 0: drain remaining projection chunks ---
                # fill order is deadline-driven: V tile t is needed at step
                # t, K qc2/qc3 at steps 8/12, Q qc2-3 only at step 64.
                with tc.tile_pool(name="ps1b", bufs=1, space="PSUM") as ps1b:
                    # V tiles 0-3 off the exp(0) critical path (PE order:
                    # after S(0), before step 0's U matmuls)
                    for t in range(4):
                        v_proj(ps1b, t)

                    def K(qc, m):
                        return lambda: qk_chunk(ps1b, wk_sb, bk_sb, kt_sb, qc, m)

                    def Q(qc, m):
                        return lambda: qk_chunk(ps1b, wq_sb, bq_sb, qt_sb, qc, m)

                    def V(t):
                        return lambda: v_proj(ps1b, t)

                    # per-step fill schedule. deadlines (fill slot i runs
                    # after the S(i+1) prefetch): K qc_j m0 by slot 4j-2,
                    # V_t by slot t, m=1 chunks by ~slot 30+4j-2, Q qc2/3
                    # m0 by slot 62, m1 by slot 94.
                    def KS(qc, m, h):
                        return lambda: qk_sub(ps1b, wk_sb, bk_sb, kt_sb, qc, m, h)

                    def QS(qc, m, h):
                        return lambda: qk_sub(ps1b, wq_sb, bq_sb, qt_sb, qc, m, h)

                    def VS(t, h):
                        return lambda: v_sub(ps1b, t, h)

                    fill_sched = {
                        0: [VS(4, 0), KS(1, 0, 0)],
                        1: [VS(4, 1), KS(1, 0, 1)],
                        2: [VS(5, 0), VS(5, 1)],
                        3: [VS(6, 0), KS(2, 0, 0)],
                        4: [VS(6, 1), VS(7, 0)],
                        5: [VS(7, 1), KS(2, 0, 1)],
                        6: [VS(8, 0), VS(8, 1)],
                        7: [VS(9, 0), KS(3, 0, 0)],
                        8: [VS(9, 1), VS(10, 0)],
                        9: [VS(10, 1), KS(3, 0, 1)],
                        10: [VS(11, 0), VS(11, 1)],
                        11: [VS(12, 0)],
                        12: [VS(12, 1), VS(13, 0)],
                        13: [VS(13, 1)],
                        14: [VS(14, 0), VS(14, 1)],
                        15: [VS(15, 0), VS(15, 1)],
                        16: [KS(0, 1, 0)],
                        17: [KS(0, 1, 1)],
                        18: [QS(0, 1, 0)],
                        19: [QS(0, 1, 1)],
                        20: [QS(1, 1, 0)],
                        21: [QS(1, 1, 1)],
                        22: [KS(1, 1, 0)],
                        23: [KS(1, 1, 1)],
                        26: [KS(2, 1, 0)],
                        27: [KS(2, 1, 1)],
                        30: [KS(3, 1, 0)],
                        31: [KS(3, 1, 1)],
                        40: [QS(2, 0, 0)],
                        41: [QS(2, 0, 1)],
                        44: [QS(3, 0, 0)],
                        45: [QS(3, 0, 1)],
                        50: [QS(2, 1, 0)],
                        51: [QS(2, 1, 1)],
                        54: [QS(3, 1, 0)],
                        55: [QS(3, 1, 1)],
                    }
                    for i in range(nsteps // 2):
                        fs = fill_sched.get(i)
                        do_step(
                            i,
                            (lambda fs=fs: [f() for f in fs]) if fs else None,
                        )

                # --- T-half 1: transpose T-half-0 norms, then interleave
                # phase-3 t-tiles 0-7 into PE slack ---
                with (
                    tc.tile_pool(name="ps_t", bufs=1, space="PSUM") as pst,
                    tc.tile_pool(name="ps_o", bufs=1, space="PSUM") as pso,
                ):
                    fills2 = []
                    for m in range(2):
                        for lo in (0, 4):
                            fills2.append(
                                lambda m=m, lo=lo: transpose_quad(pst, m, 0, lo)
                            )
                    for t in range(8):
                        for n in range(2):
                            fills2.append(lambda t=t, n=n: phase3_half(pso, t, n))
                    fi = 0
                    for i in range(nsteps // 2, nsteps):
                        th, h, kt = steps[i]
                        fill = None
                        if fi < 4 or (fi < len(fills2) and (i - 64) % 3 == 2):
                            fill = fills2[fi]
                            fi += 1
                        do_step(i, fill)
                        if kt == KT - 1 and h % 2 == 1 and h < NHC - 1:
                            # pair-0 T-half-1 norms ready: transpose them
                            transpose_quad(pst, h // 2, 1, 0)
                            transpose_quad(pst, h // 2, 1, 4)
                    for f in fills2[fi:]:
                        f()
                    # last pair (m=1, th=1) transposes on the critical tail
                    with tc.high_priority(offset=40):
                        transpose_quad(pst, 1, 1, 0)
                        transpose_quad(pst, 1, 1, 4)

            # ---- phase 3 tail: t-tiles 8-15 (phase-2 pools closed) ----
            with tc.tile_pool(name="ps_ot", bufs=4, space="PSUM") as psot:
                for t in range(8, KT):
                    for n in range(2):
                        phase3_half(
                            psot, t, n,
                            copy_engine=nc.scalar if (t + n) % 2 else nc.vector,
                        )
    nc.compile()
    return nc


def _get_nc(repeat=1):
    key = ("nc", repeat)
    if key not in _CACHE:
        _CACHE[key] = _build(repeat)
    return _CACHE[key]


def _prep_core_inputs(c, x, mask, Wq, bq, Wk, bk, Wv, bv, Wo):
    b, hb = divmod(c, NCORES // B)
    sl = slice(hb * C, (hb + 1) * C)
    return {
        "xt": np.ascontiguousarray(x[b].T).astype(np.float16),
        "wq": np.ascontiguousarray(Wq[:, sl] * SCALE).astype(np.float16),
        "wk": np.ascontiguousarray(Wk[:, sl]).astype(np.float16),
        "wv": np.ascontiguousarray(Wv[:, sl]).astype(np.float16),
        "wo": np.ascontiguousarray(Wo[sl, :]).astype(np.float16),
        "maskt": np.ascontiguousarray(mask[b].T).astype(np.float16),
        "bqs": (bq[sl] * SCALE).astype(np.float32),
        "bks": bk[sl].astype(np.float32),
        "bvr": bv[sl].astype(np.float16).reshape(1, C),
        "ident": np.eye(128, dtype=np.float16),
    }


def kernel(
    inputs, mask, Wq, bq, Wk, bk, Wv, bv, Wo, bo,
    _trace=False, _trace_kwargs=None, _repeat=1,
):
    x = np.asarray(inputs, dtype=np.float32)
    mask = np.asarray(mask)
    Wq, bq = np.asarray(Wq, np.float32), np.asarray(bq, np.float32)
    Wk, bk = np.asarray(Wk, np.float32), np.asarray(bk, np.float32)
    Wv, bv = np.asarray(Wv, np.float32), np.asarray(bv, np.float32)
    Wo, bo = np.asarray(Wo, np.float32), np.asarray(bo, np.float32)

    nc = _get_nc(_repeat)
    in_maps = [
        _prep_core_inputs(c, x, mask, Wq, bq, Wk, bk, Wv, bv, Wo)
        for c in range(NCORES)
    ]
    last_err = None
    for attempt in range(3):
        try:
            res = run_bass_kernel_spmd(
                nc,
                in_maps,
                list(range(NCORES)),
                trace=_trace,
                **(_trace_kwargs or {}),
            )
            break
        except Exception as e:  # wedged device etc. -- retry
            last_err = e
            time.sleep(3.0)
    else:
        raise last_err
    out = np.empty((B, T, D), np.float32)
    per_b = NCORES // B
    for b in range(B):
        acc = res.results[b * per_b]["out"].astype(np.float32)
        for j in range(1, per_b):
            acc = acc + res.results[b * per_b + j]["out"].astype(np.float32)
        out[b] = acc + bo[None, :]
    if _trace:
        kernel.last_results = res
    return out


# revision 32
# speedup vs baseline: 1.0121x; 1.0121x over previous
"""Multi-head attention on 8 Trainium2 NeuronCores.

Problem: B=2, T=2048, D=1024, H=16 heads (dh=64), int 0/1 attention mask.

Sharding (hardcoded): core c -> batch b = c//4, head block hb = c%4
(4 heads = 256 cols per core). Wq/Wk/Wv column-sharded, Wo row-sharded;
each core returns a partial [T, D] output, host sums the 4 partials per
batch and adds bo.

Per-core kernel (all matmul inputs fp16, fp32 accumulation):
  phase 1: K^T/Q^T projections (scale folded into Wq/bq on the host, bias
           added during the PSUM->SBUF copy on DVE via tensor_scalar add).
           Only (K,Q) x (qc0,qc1) + V tiles 0-3 run up front; the
           remaining chunks drain one-per-step into phase-2 PE slack.
  phase 2: steps ordered (T-half, head, kt).  Per step:
           S^T[k, q-half] = K_h[kt] Q_h^T   ([128,1024] PSUM)
           E = exp(S^T) (ACT)  ->  E *= mask^T tile (DVE, fp16 2x)
           per q-tile: U[q,0:64] += E_tile^T V_h[kt]  (E stationary, n=64)
                       den[q]    += E_tile^T ones     (n=1)
           at kt=15: DVE reciprocal(den) + per-q-tile tensor_scalar_mul
           -> norm tile [q, pair, dh]; per head-pair DMA-xbar transpose
           [128q,128c] -> hc[c, q].
  phase 3: O_partial = hc^T.T Wo.  t-tiles 0-7 interleave into the second
           T-half's PE slack; t-tiles 8-15 run in the tail.  DMA out fp16
           (summed in f32 on the host).

No max-subtraction is needed: scores are O(1) and the masked-
multiplicative form E = exp(S) * m / sum(exp(S) * m) is exact.
"""
import contextlib
import os
import sys
import time

os.environ.setdefault("NEURON_RT_RESET_CORES", "1")

if "/opt/trn_rl_repo" not in sys.path:
    sys.path.insert(0, "/opt/trn_rl_repo")

import numpy as np

import concourse.bass as bass  # noqa: F401  (import keeps bass registered)
from concourse import bacc
import concourse.mybir as mybir
import concourse.tile as tile
from concourse.bass_utils import run_bass_kernel_spmd

f32 = mybir.dt.float32
f16 = mybir.dt.float16
AF = mybir.ActivationFunctionType

B, T, D, H = 2, 2048, 1024, 16
DH = 64                 # head dim
NHC = 4                 # heads per core
C = NHC * DH            # 256 columns per core
KD = D // 128           # 8 contraction tiles over D
KT = T // 128           # 16 k-tiles over T
QC = T // 512           # 4 q chunks of 512
QT = T // 128           # 16 q tiles of 128
NCORES = 8
SCALE = DH ** -0.5      # 0.125

_CACHE = {}


def _build(repeat=1):
    nc = bacc.Bacc()
    xt = nc.declare_dram_parameter("xt", [D, T], f16, isOutput=False)
    wq = nc.declare_dram_parameter("wq", [D, C], f16, isOutput=False)
    wk = nc.declare_dram_parameter("wk", [D, C], f16, isOutput=False)
    wv = nc.declare_dram_parameter("wv", [D, C], f16, isOutput=False)
    wo = nc.declare_dram_parameter("wo", [C, D], f16, isOutput=False)
    maskt = nc.declare_dram_parameter("maskt", [T, T], f16, isOutput=False)
    bqs = nc.declare_dram_parameter("bqs", [C], f32, isOutput=False)
    bks = nc.declare_dram_parameter("bks", [C], f32, isOutput=False)
    bvr = nc.declare_dram_parameter("bvr", [1, C], f16, isOutput=False)
    ident = nc.declare_dram_parameter("ident", [128, 128], f16, isOutput=False)
    out = nc.declare_dram_parameter("out", [T, D], f16, isOutput=True)

    with tile.TileContext(nc) as tc:
        loop_ctx = tc.For_i(0, repeat, 1) if repeat > 1 else contextlib.nullcontext()
        with (
            loop_ctx,
            tc.tile_pool(name="persist", bufs=1) as pp,
            tc.tile_pool(name="e", bufs=10) as ep,
            tc.tile_pool(name="norm", bufs=2) as npool,
            tc.tile_pool(name="osb", bufs=4) as op_,
            tc.tile_pool(name="small", bufs=4) as sp,
        ):
            xt_sb = pp.tile([128, KD, T], f16)
            wq_sb = pp.tile([128, KD, C], f16)
            wk_sb = pp.tile([128, KD, C], f16)
            wv_sb = pp.tile([128, KD, C], f16)
            wo_sb = pp.tile([128, C // 128, D], f16)
            mk_sb = pp.tile([128, KT, T], f16)
            qt_sb = pp.tile([128, C // 128, T], f16)
            kt_sb = pp.tile([128, C // 128, T], f16)
            v_sb = pp.tile([128, KT, NHC, DH + 1], f16)
            hc_sb = pp.tile([128, C // 128, T], f16)
            bq_sb = pp.tile([128, C // 128], f32)
            bk_sb = pp.tile([128, C // 128], f32)
            bv_sb = pp.tile([1, C], f16)
            ones128 = pp.tile([1, 128], f16)
            id_sb = pp.tile([128, 128], f16)

            # ---- input DMAs ----
            xt_r = xt.rearrange("(kd p) t -> p kd t", p=128)
            wq_r = wq.rearrange("(kd p) c -> p kd c", p=128)
            wk_r = wk.rearrange("(kd p) c -> p kd c", p=128)
            wv_r = wv.rearrange("(kd p) c -> p kd c", p=128)
            # DMA transfers serialize on one device (~bytes/360GB/s);
            # SWDGE (gpsimd) DMAs issue at t=0 and would interleave with the
            # startup-critical stream, so EVERYTHING goes on the one sync
            # HWDGE queue in exact deadline order.
            mk_r = maskt.rearrange("(kt p) t -> p kt t", p=128)
            nc.sync.dma_start(out=wk_sb[:, 0:2, :], in_=wk_r[:, 0:2, :])
            nc.sync.dma_start(out=wk_sb[:, 2:KD, :], in_=wk_r[:, 2:KD, :])
            for kd2 in range(0, KD, 2):
                nc.sync.dma_start(
                    out=xt_sb[:, kd2 : kd2 + 2, 0:512],
                    in_=xt_r[:, kd2 : kd2 + 2, 0:512],
                )
            nc.sync.dma_start(out=bk_sb, in_=bks.rearrange("(m p) -> p m", p=128))
            nc.sync.dma_start(out=bq_sb, in_=bqs.rearrange("(m p) -> p m", p=128))
            nc.sync.dma_start(out=wq_sb[:, 0:4, :], in_=wq_r[:, 0:4, :])
            nc.sync.dma_start(out=wq_sb[:, 4:KD, :], in_=wq_r[:, 4:KD, :])
            for kd4 in range(0, KD, 4):
                nc.sync.dma_start(
                    out=xt_sb[:, kd4 : kd4 + 4, 512:1024],
                    in_=xt_r[:, kd4 : kd4 + 4, 512:1024],
                )
            nc.sync.dma_start(out=wv_sb, in_=wv_r)
            nc.sync.dma_start(out=bv_sb, in_=bvr[:, :])
            for kt in range(0, 6):
                nc.sync.dma_start(out=mk_sb[:, kt, :], in_=mk_r[:, kt, :])
            nc.sync.dma_start(
                out=xt_sb[:, :, 1024:1536], in_=xt_r[:, :, 1024:1536]
            )
            for kt in range(6, 9):
                nc.sync.dma_start(out=mk_sb[:, kt, :], in_=mk_r[:, kt, :])
            nc.sync.dma_start(
                out=xt_sb[:, :, 1536:2048], in_=xt_r[:, :, 1536:2048]
            )
            for kt in range(9, KT):
                nc.sync.dma_start(out=mk_sb[:, kt, :], in_=mk_r[:, kt, :])
            nc.sync.dma_start(out=wo_sb, in_=wo.rearrange("(m p) d -> p m d", p=128))
            nc.sync.dma_start(out=id_sb, in_=ident[:, :])
            nc.vector.memset(ones128, 1.0)
            nc.vector.memset(v_sb[:, :, :, DH : DH + 1], 1.0)

            # ---- projection chunk helpers ----
            def qk_sub(pool, w_sb, b_sb, dst, qc, m, half, state={}):
                # half 0 emits kd 0-3 (allocates the PSUM tile), half 1 emits
                # kd 4-7 + the bias/convert copy -- lets a 1.7us chunk drain
                # across two fill slots
                if half == 0:
                    state[(qc, m)] = pool.tile([128, 512], f32, tag="p", name="pt")
                pt = state[(qc, m)]
                for kd in range(half * 4, half * 4 + 4):
                    nc.tensor.matmul(
                        pt,
                        w_sb[:, kd, m * 128 : (m + 1) * 128],
                        xt_sb[:, kd, qc * 512 : (qc + 1) * 512],
                        start=(kd == 0),
                        stop=(kd == KD - 1),
                    )
                if half == 1:
                    state.pop((qc, m))
                    # bias add + f16 convert on DVE (bias/scale pre-folded on
                    # host); broadcast add avoids TensorScalarPtr SEQ cost
                    nc.vector.tensor_tensor(
                        dst[:, m, qc * 512 : (qc + 1) * 512],
                        pt,
                        b_sb[:, m : m + 1].to_broadcast((128, 512)),
                        mybir.AluOpType.add,
                    )

            def qk_chunk(pool, w_sb, b_sb, dst, qc, m):
                qk_sub(pool, w_sb, b_sb, dst, qc, m, 0)
                qk_sub(pool, w_sb, b_sb, dst, qc, m, 1)

            def v_sub(pool, t, half, state={}):
                if half == 0:
                    state[t] = pool.tile([128, C], f32, tag="v", name="pv")
                pv = state[t]
                for kd in range(half * 4, half * 4 + 4):
                    nc.tensor.matmul(
                        pv,
                        xt_sb[:, kd, t * 128 : (t + 1) * 128],
                        wv_sb[:, kd, :],
                        start=(kd == 0),
                        stop=False,
                    )
                if half == 1:
                    state.pop(t)
                    nc.tensor.matmul(pv, ones128, bv_sb, start=False, stop=True)
                    nc.vector.tensor_copy(
                        v_sb[:, t, :, 0:DH],
                        pv.rearrange("p (h x) -> p h x", x=DH),
                    )

            def v_proj(pool, t):
                v_sub(pool, t, 0)
                v_sub(pool, t, 1)

            # ---- phase 1 (startup): the minimum for step (th0,h0,kt0) ----
            # m=1 chunks (head pair 2/3) are only needed from step 32, V and
            # the remaining K/Q chunks drain into phase-2 PE slack (a fill
            # feeding step i MUST be emitted at a fill slot <= i: the tile
            # scheduler keeps per-engine emission order).
            with tc.tile_pool(name="ps1", bufs=4, space="PSUM") as ps1:
                # PE p-state warmup on zeros during the DMA lead (the tensor
                # engine ramps 0.65->2.4GHz over ~3us of continuous work),
                # and pre-load the Exp activation table off the critical path
                junk = sp.tile([128, 512], f16, tag="junk", name="junk")
                escr = sp.tile([1, 128], f16, tag="escr", name="escr")
                nc.vector.memset(junk, 0.0)
                nc.vector.memset(escr, 0.0)
                nc.scalar.activation(escr, escr, AF.Exp)
                wut = ps1.tile([128, 512], f32, tag="p", name="wut")
                for w in range(9):
                    nc.tensor.matmul(
                        wut, junk[:, 0:128], junk, start=(w == 0), stop=(w == 8)
                    )
                qk_chunk(ps1, wk_sb, bk_sb, kt_sb, 0, 0)
                qk_chunk(ps1, wq_sb, bq_sb, qt_sb, 0, 0)
                qk_chunk(ps1, wq_sb, bq_sb, qt_sb, 1, 0)

            # ---- phase 2 ----
            steps = [
                (th, h, kt)
                for th in range(2)
                for h in range(NHC)
                for kt in range(KT)
            ]
            nsteps = len(steps)

            with (
                tc.tile_pool(name="ps_s", bufs=2, space="PSUM") as pss,
                tc.tile_pool(name="ps_u", bufs=1, space="PSUM") as psu,
            ):
                def s_matmuls(th, h, kt):
                    m, p0 = h // 2, (h % 2) * 64
                    with tc.high_priority(offset=24):
                        st = pss.tile([128, 1024], f32, tag="s")
                        for sub in range(2):
                            q0 = th * 1024 + sub * 512
                            nc.tensor.matmul(
                                st[:, sub * 512 : (sub + 1) * 512],
                                kt_sb[p0 : p0 + 64, m, kt * 128 : (kt + 1) * 128],
                                qt_sb[p0 : p0 + 64, m, q0 : q0 + 512],
                                start=True,
                                stop=True,
                            )
                    return st

                norm_tiles = {}
                u = None

                p3_state = {}

                def phase3_half(pool, t, n, copy_engine=None):
                    if n == 0:
                        p3_state[t] = op_.tile([128, 1024], f16, name="ot")
                    ot = p3_state[t]
                    po = pool.tile([128, 512], f32, tag="o")
                    for m in range(C // 128):
                        nc.tensor.matmul(
                            po,
                            hc_sb[:, m, t * 128 : (t + 1) * 128],
                            wo_sb[:, m, n * 512 : (n + 1) * 512],
                            start=(m == 0),
                            stop=(m == C // 128 - 1),
                        )
                    eng = copy_engine or nc.vector
                    if eng is nc.scalar:
                        nc.scalar.activation(
                            ot[:, n * 512 : (n + 1) * 512], po, AF.Identity
                        )
                    else:
                        eng.tensor_copy(ot[:, n * 512 : (n + 1) * 512], po)
                    if n == 1:
                        p3_state.pop(t)
                        nc.sync.dma_start(
                            out=out[t * 128 : (t + 1) * 128, :], in_=ot
                        )

                def transpose_quad(pst, m, th, lo):
                    # PE xbar-free transpose of 4 [128q,128c] norm tiles into
                    # hc layout; Pool copies them out of PSUM.
                    nt = norm_tiles[m]
                    tp = pst.tile([128, 8, 128], f16, tag="t", name="tp")
                    for j in range(4):
                        qt = th * 8 + lo + j
                        nc.tensor.matmul(
                            tp[:, lo + j, :],
                            nt[:, qt, :, :],
                            id_sb,
                            is_transpose=True,
                        )
                        nc.vector.tensor_copy(
                            hc_sb[:, m, qt * 128 : (qt + 1) * 128],
                            tp[:, lo + j, :],
                        )

                def do_step(i, fill_fn):
                    nonlocal u
                    th, h, kt = steps[i]
                    m = h // 2
                    if kt == 0:
                        # [q, 8*64 U | pad | 8 denom]  (2 PSUM banks)
                        u = psu.tile([128, 528], f32, tag="u")
                    st_cur = do_step.st_next
                    if i + 1 < nsteps:
                        do_step.st_next = s_matmuls(*steps[i + 1])
                    if fill_fn is not None:
                        fill_fn()
                    e = ep.tile([128, 1024], f16)
                    nc.scalar.activation(e, st_cur, AF.Exp)
                    nc.vector.tensor_mul(
                        e, e, mk_sb[:, kt, th * 1024 : (th + 1) * 1024]
                    )
                    # PSUM zero-regions are 2KB banks: only the first chain
                    # in each bank may set start=True (it pending-zeroes the
                    # whole bank); sibling chains at kt=0 then overwrite via
                    # the pending-zero flags.
                    for qtl in range(8):
                        nc.tensor.matmul(
                            u[:, qtl * DH : (qtl + 1) * DH],
                            e[:, qtl * 128 : (qtl + 1) * 128],
                            v_sb[:, kt, h, 0:DH],
                            start=(kt == 0 and qtl == 0),
                            stop=(kt == KT - 1),
                            skip_group_check=True,
                        )
                        nc.tensor.matmul(
                            u[:, 512 + qtl : 513 + qtl],
                            e[:, qtl * 128 : (qtl + 1) * 128],
                            v_sb[:, kt, h, DH : DH + 1],
                            start=(kt == 0 and qtl == 0),
                            stop=(kt == KT - 1),
                            skip_group_check=True,
                        )
                    if kt == KT - 1:
                        with tc.high_priority(offset=40):
                            if m not in norm_tiles:
                                norm_tiles[m] = npool.tile(
                                    [128, QT, 2, DH], f16, tag="n", name=f"nt{m}"
                                )
                            nt = norm_tiles[m]
                            rec = sp.tile([128, 8, 1], f32, tag="r")
                            nc.vector.reciprocal(
                                rec[:, :, 0], u[:, 512:520]
                            )
                            # single broadcast multiply (recip varies along
                            # the qt free dim -> 0-stride broadcast over dh)
                            nc.vector.tensor_tensor(
                                nt[:, th * 8 : (th + 1) * 8, h % 2, :],
                                u[:, 0:512].rearrange("p (a b) -> p a b", b=DH),
                                rec.to_broadcast((128, 8, DH)),
                                mybir.AluOpType.mult,
                            )

                do_step.st_next = s_matmuls(*steps[0])

                # --- T-half 0: drain remaining projection chunks ---
                # fill order is deadline-driven: V tile t is needed at step
                # t, K qc2/qc3 at steps 8/12, Q qc2-3 only at step 64.
                with tc.tile_pool(name="ps1b", bufs=1, space="PSUM") as ps1b:
                    # V tiles 0-3 off the exp(0) critical path (PE order:
                    # after S(0), before step 0's U matmuls)
                    for t in range(4):
                        v_proj(ps1b, t)

                    def K(qc, m):
                        return lambda: qk_chunk(ps1b, wk_sb, bk_sb, kt_sb, qc, m)

                    def Q(qc, m):
                        return lambda: qk_chunk(ps1b, wq_sb, bq_sb, qt_sb, qc, m)

                    def V(t):
                        return lambda: v_proj(ps1b, t)

                    # per-step fill schedule. deadlines (fill slot i runs
                    # after the S(i+1) prefetch): K qc_j m0 by slot 4j-2,
                    # V_t by slot t, m=1 chunks by ~slot 30+4j-2, Q qc2/3
                    # m0 by slot 62, m1 by slot 94.
                    def KS(qc, m, h):
                        return lambda: qk_sub(ps1b, wk_sb, bk_sb, kt_sb, qc, m, h)

                    def QS(qc, m, h):
                        return lambda: qk_sub(ps1b, wq_sb, bq_sb, qt_sb, qc, m, h)

                    def VS(t, h):
                        return lambda: v_sub(ps1b, t, h)

                    fill_sched = {
                        0: [VS(4, 0), KS(1, 0, 0)],
                        1: [VS(4, 1), KS(1, 0, 1)],
                        2: [VS(5, 0), VS(5, 1)],
                        3: [VS(6, 0), KS(2, 0, 0)],
                        4: [VS(6, 1), VS(7, 0)],
                        5: [VS(7, 1), KS(2, 0, 1)],
                        6: [VS(8, 0), VS(8, 1)],
                        7: [VS(9, 0), KS(3, 0, 0)],
                        8: [VS(9, 1), VS(10, 0)],
                        9: [VS(10, 1), KS(3, 0, 1)],
                        10: [VS(11, 0), VS(11, 1)],
                        11: [VS(12, 0)],
                        12: [VS(12, 1), VS(13, 0)],
                        13: [VS(13, 1)],
                        14: [VS(14, 0), VS(14, 1)],
                        15: [VS(15, 0), VS(15, 1)],
                        16: [KS(0, 1, 0)],
                        18: [KS(0, 1, 1)],
                        20: [QS(0, 1, 0)],
                        22: [QS(0, 1, 1)],
                        24: [QS(1, 1, 0)],
                        26: [QS(1, 1, 1)],
                        28: [KS(1, 1, 0)],
                        30: [KS(1, 1, 1)],
                        32: [KS(2, 1, 0)],
                        34: [KS(2, 1, 1)],
                        36: [KS(3, 1, 0)],
                        38: [KS(3, 1, 1)],
                        40: [QS(2, 0, 0)],
                        42: [QS(2, 0, 1)],
                        44: [QS(3, 0, 0)],
                        46: [QS(3, 0, 1)],
                        48: [QS(2, 1, 0)],
                        50: [QS(2, 1, 1)],
                        52: [QS(3, 1, 0)],
                        54: [QS(3, 1, 1)],
                    }
                    for i in range(nsteps // 2):
                        fs = fill_sched.get(i)
                        do_step(
                            i,
                            (lambda fs=fs: [f() for f in fs]) if fs else None,
                        )

                # --- T-half 1: transpose T-half-0 norms, then interleave
                # phase-3 t-tiles 0-7 into PE slack ---
                with (
                    tc.tile_pool(name="ps_t", bufs=1, space="PSUM") as pst,
                    tc.tile_pool(name="ps_o", bufs=1, space="PSUM") as pso,
                ):
                    fills2 = []
                    for m in range(2):
                        for lo in (0, 4):
                            fills2.append(
                                lambda m=m, lo=lo: transpose_quad(pst, m, 0, lo)
                            )
                    for t in range(8):
                        for n in range(2):
                            fills2.append(lambda t=t, n=n: phase3_half(pso, t, n))
                    fi = 0
                    for i in range(nsteps // 2, nsteps):
                        th, h, kt = steps[i]
                        fill = None
                        if fi < 4 or (fi < len(fills2) and (i - 64) % 3 == 2):
                            fill = fills2[fi]
                            fi += 1
                        do_step(i, fill)
                        if kt == KT - 1 and h % 2 == 1 and h < NHC - 1:
                            # pair-0 T-half-1 norms ready: transpose them
                            transpose_quad(pst, h // 2, 1, 0)
                            transpose_quad(pst, h // 2, 1, 4)
                    for f in fills2[fi:]:
                        f()
                    # last pair (m=1, th=1) transposes on the critical tail
                    with tc.high_priority(offset=40):
                        transpose_quad(pst, 1, 1, 0)
                        transpose_quad(pst, 1, 1, 4)

            # ---- phase 3 tail: t-tiles 8-15 (phase-2 pools closed) ----
            with tc.tile_pool(name="ps_ot", bufs=4, space="PSUM") as psot:
                for t in range(8, KT):
                    for n in range(2):
                        phase3_half(
                            psot, t, n,
                            copy_engine=nc.scalar if (t + n) % 2 else nc.vector,
                        )
    nc.compile()
    return nc


def _get_nc(repeat=1):
    key = ("nc", repeat)
    if key not in _CACHE:
        _CACHE[key] = _build(repeat)
    return _CACHE[key]


def _prep_core_inputs(c, x, mask, Wq, bq, Wk, bk, Wv, bv, Wo):
    b, hb = divmod(c, NCORES // B)
    sl = slice(hb * C, (hb + 1) * C)
    return {
        "xt": np.ascontiguousarray(x[b].T).astype(np.float16),
        "wq": np.ascontiguousarray(Wq[:, sl] * SCALE).astype(np.float16),
        "wk": np.ascontiguousarray(Wk[:, sl]).astype(np.float16),
        "wv": np.ascontiguousarray(Wv[:, sl]).astype(np.float16),
        "wo": np.ascontiguousarray(Wo[sl, :]).astype(np.float16),
        "maskt": np.ascontiguousarray(mask[b].T).astype(np.float16),
        "bqs": (bq[sl] * SCALE).astype(np.float32),
        "bks": bk[sl].astype(np.float32),
        "bvr": bv[sl].astype(np.float16).reshape(1, C),
        "ident": np.eye(128, dtype=np.float16),
    }


def kernel(
    inputs, mask, Wq, bq, Wk, bk, Wv, bv, Wo, bo,
    _trace=False, _trace_kwargs=None, _repeat=1,
):
    x = np.asarray(inputs, dtype=np.float32)
    mask = np.asarray(mask)
    Wq, bq = np.asarray(Wq, np.float32), np.asarray(bq, np.float32)
    Wk, bk = np.asarray(Wk, np.float32), np.asarray(bk, np.float32)
    Wv, bv = np.asarray(Wv, np.float32), np.asarray(bv, np.float32)
    Wo, bo = np.asarray(Wo, np.float32), np.asarray(bo, np.float32)

    nc = _get_nc(_repeat)
    in_maps = [
        _prep_core_inputs(c, x, mask, Wq, bq, Wk, bk, Wv, bv, Wo)
        for c in range(NCORES)
    ]
    last_err = None
    for attempt in range(3):
        try:
            res = run_bass_kernel_spmd(
                nc,
                in_maps,
                list(range(NCORES)),
                trace=_trace,
                **(_trace_kwargs or {}),
            )
            break
        except Exception as e:  # wedged device etc. -- retry
            last_err = e
            time.sleep(3.0)
    else:
        raise last_err
    out = np.empty((B, T, D), np.float32)
    per_b = NCORES // B
    for b in range(B):
        acc = res.results[b * per_b]["out"].astype(np.float32)
        for j in range(1, per_b):
            acc = acc + res.results[b * per_b + j]["out"].astype(np.float32)
        out[b] = acc + bo[None, :]
    if _trace:
        kernel.last_results = res
    return out


# revision 33
# speedup vs baseline: 1.0140x; 1.0018x over previous
"""Multi-head attention on 8 Trainium2 NeuronCores.

Problem: B=2, T=2048, D=1024, H=16 heads (dh=64), int 0/1 attention mask.

Sharding (hardcoded): core c -> batch b = c//4, head block hb = c%4
(4 heads = 256 cols per core). Wq/Wk/Wv column-sharded, Wo row-sharded;
each core returns a partial [T, D] output, host sums the 4 partials per
batch and adds bo.

Per-core kernel (all matmul inputs fp16, fp32 accumulation):
  phase 1: K^T/Q^T projections (scale folded into Wq/bq on the host, bias
           added during the PSUM->SBUF copy on DVE via tensor_scalar add).
           Only (K,Q) x (qc0,qc1) + V tiles 0-3 run up front; the
           remaining chunks drain one-per-step into phase-2 PE slack.
  phase 2: steps ordered (T-half, head, kt).  Per step:
           S^T[k, q-half] = K_h[kt] Q_h^T   ([128,1024] PSUM)
           E = exp(S^T) (ACT)  ->  E *= mask^T tile (DVE, fp16 2x)
           per q-tile: U[q,0:64] += E_tile^T V_h[kt]  (E stationary, n=64)
                       den[q]    += E_tile^T ones     (n=1)
           at kt=15: DVE reciprocal(den) + per-q-tile tensor_scalar_mul
           -> norm tile [q, pair, dh]; per head-pair DMA-xbar transpose
           [128q,128c] -> hc[c, q].
  phase 3: O_partial = hc^T.T Wo.  t-tiles 0-7 interleave into the second
           T-half's PE slack; t-tiles 8-15 run in the tail.  DMA out fp16
           (summed in f32 on the host).

No max-subtraction is needed: scores are O(1) and the masked-
multiplicative form E = exp(S) * m / sum(exp(S) * m) is exact.
"""
import contextlib
import os
import sys
import time

os.environ.setdefault("NEURON_RT_RESET_CORES", "1")

if "/opt/trn_rl_repo" not in sys.path:
    sys.path.insert(0, "/opt/trn_rl_repo")

import numpy as np

import concourse.bass as bass  # noqa: F401  (import keeps bass registered)
from concourse import bacc
import concourse.mybir as mybir
import concourse.tile as tile
from concourse.bass_utils import run_bass_kernel_spmd

f32 = mybir.dt.float32
f16 = mybir.dt.float16
AF = mybir.ActivationFunctionType

B, T, D, H = 2, 2048, 1024, 16
DH = 64                 # head dim
NHC = 4                 # heads per core
C = NHC * DH            # 256 columns per core
KD = D // 128           # 8 contraction tiles over D
KT = T // 128           # 16 k-tiles over T
QC = T // 512           # 4 q chunks of 512
QT = T // 128           # 16 q tiles of 128
NCORES = 8
SCALE = DH ** -0.5      # 0.125

_CACHE = {}


def _build(repeat=1):
    nc = bacc.Bacc()
    xt = nc.declare_dram_parameter("xt", [D, T], f16, isOutput=False)
    wq = nc.declare_dram_parameter("wq", [D, C], f16, isOutput=False)
    wk = nc.declare_dram_parameter("wk", [D, C], f16, isOutput=False)
    wv = nc.declare_dram_parameter("wv", [D, C], f16, isOutput=False)
    wo = nc.declare_dram_parameter("wo", [C, D], f16, isOutput=False)
    maskt = nc.declare_dram_parameter("maskt", [T, T], f16, isOutput=False)
    bqs = nc.declare_dram_parameter("bqs", [C], f32, isOutput=False)
    bks = nc.declare_dram_parameter("bks", [C], f32, isOutput=False)
    bvr = nc.declare_dram_parameter("bvr", [1, C], f16, isOutput=False)
    ident = nc.declare_dram_parameter("ident", [128, 128], f16, isOutput=False)
    out = nc.declare_dram_parameter("out", [T, D], f16, isOutput=True)

    with tile.TileContext(nc) as tc:
        loop_ctx = tc.For_i(0, repeat, 1) if repeat > 1 else contextlib.nullcontext()
        with (
            loop_ctx,
            tc.tile_pool(name="persist", bufs=1) as pp,
            tc.tile_pool(name="e", bufs=10) as ep,
            tc.tile_pool(name="norm", bufs=2) as npool,
            tc.tile_pool(name="osb", bufs=6) as op_,
            tc.tile_pool(name="small", bufs=4) as sp,
        ):
            xt_sb = pp.tile([128, KD, T], f16)
            wq_sb = pp.tile([128, KD, C], f16)
            wk_sb = pp.tile([128, KD, C], f16)
            wv_sb = pp.tile([128, KD, C], f16)
            wo_sb = pp.tile([128, C // 128, D], f16)
            mk_sb = pp.tile([128, KT, T], f16)
            qt_sb = pp.tile([128, C // 128, T], f16)
            kt_sb = pp.tile([128, C // 128, T], f16)
            v_sb = pp.tile([128, KT, NHC, DH + 1], f16)
            hc_sb = pp.tile([128, C // 128, T], f16)
            bq_sb = pp.tile([128, C // 128], f32)
            bk_sb = pp.tile([128, C // 128], f32)
            bv_sb = pp.tile([1, C], f16)
            ones128 = pp.tile([1, 128], f16)
            id_sb = pp.tile([128, 128], f16)

            # ---- input DMAs ----
            xt_r = xt.rearrange("(kd p) t -> p kd t", p=128)
            wq_r = wq.rearrange("(kd p) c -> p kd c", p=128)
            wk_r = wk.rearrange("(kd p) c -> p kd c", p=128)
            wv_r = wv.rearrange("(kd p) c -> p kd c", p=128)
            # DMA transfers serialize on one device (~bytes/360GB/s);
            # SWDGE (gpsimd) DMAs issue at t=0 and would interleave with the
            # startup-critical stream, so EVERYTHING goes on the one sync
            # HWDGE queue in exact deadline order.
            mk_r = maskt.rearrange("(kt p) t -> p kt t", p=128)
            nc.sync.dma_start(out=wk_sb[:, 0:2, :], in_=wk_r[:, 0:2, :])
            nc.sync.dma_start(out=wk_sb[:, 2:KD, :], in_=wk_r[:, 2:KD, :])
            for kd2 in range(0, KD, 2):
                nc.sync.dma_start(
                    out=xt_sb[:, kd2 : kd2 + 2, 0:512],
                    in_=xt_r[:, kd2 : kd2 + 2, 0:512],
                )
            nc.sync.dma_start(out=bk_sb, in_=bks.rearrange("(m p) -> p m", p=128))
            nc.sync.dma_start(out=bq_sb, in_=bqs.rearrange("(m p) -> p m", p=128))
            nc.sync.dma_start(out=wq_sb[:, 0:4, :], in_=wq_r[:, 0:4, :])
            nc.sync.dma_start(out=wq_sb[:, 4:KD, :], in_=wq_r[:, 4:KD, :])
            for kd4 in range(0, KD, 4):
                nc.sync.dma_start(
                    out=xt_sb[:, kd4 : kd4 + 4, 512:1024],
                    in_=xt_r[:, kd4 : kd4 + 4, 512:1024],
                )
            nc.sync.dma_start(out=wv_sb, in_=wv_r)
            nc.sync.dma_start(out=bv_sb, in_=bvr[:, :])
            for kt in range(0, 6):
                nc.sync.dma_start(out=mk_sb[:, kt, :], in_=mk_r[:, kt, :])
            nc.sync.dma_start(
                out=xt_sb[:, :, 1024:1536], in_=xt_r[:, :, 1024:1536]
            )
            for kt in range(6, 9):
                nc.sync.dma_start(out=mk_sb[:, kt, :], in_=mk_r[:, kt, :])
            nc.sync.dma_start(
                out=xt_sb[:, :, 1536:2048], in_=xt_r[:, :, 1536:2048]
            )
            for kt in range(9, KT):
                nc.sync.dma_start(out=mk_sb[:, kt, :], in_=mk_r[:, kt, :])
            nc.sync.dma_start(out=wo_sb, in_=wo.rearrange("(m p) d -> p m d", p=128))
            nc.sync.dma_start(out=id_sb, in_=ident[:, :])
            nc.vector.memset(ones128, 1.0)
            nc.vector.memset(v_sb[:, :, :, DH : DH + 1], 1.0)

            # ---- projection chunk helpers ----
            def qk_sub(pool, w_sb, b_sb, dst, qc, m, half, state={}):
                # half 0 emits kd 0-3 (allocates the PSUM tile), half 1 emits
                # kd 4-7 + the bias/convert copy -- lets a 1.7us chunk drain
                # across two fill slots
                if half == 0:
                    state[(qc, m)] = pool.tile([128, 512], f32, tag="p", name="pt")
                pt = state[(qc, m)]
                for kd in range(half * 4, half * 4 + 4):
                    nc.tensor.matmul(
                        pt,
                        w_sb[:, kd, m * 128 : (m + 1) * 128],
                        xt_sb[:, kd, qc * 512 : (qc + 1) * 512],
                        start=(kd == 0),
                        stop=(kd == KD - 1),
                    )
                if half == 1:
                    state.pop((qc, m))
                    # bias add + f16 convert on DVE (bias/scale pre-folded on
                    # host); broadcast add avoids TensorScalarPtr SEQ cost
                    nc.vector.tensor_tensor(
                        dst[:, m, qc * 512 : (qc + 1) * 512],
                        pt,
                        b_sb[:, m : m + 1].to_broadcast((128, 512)),
                        mybir.AluOpType.add,
                    )

            def qk_chunk(pool, w_sb, b_sb, dst, qc, m):
                qk_sub(pool, w_sb, b_sb, dst, qc, m, 0)
                qk_sub(pool, w_sb, b_sb, dst, qc, m, 1)

            def v_sub(pool, t, half, state={}):
                if half == 0:
                    state[t] = pool.tile([128, C], f32, tag="v", name="pv")
                pv = state[t]
                for kd in range(half * 4, half * 4 + 4):
                    nc.tensor.matmul(
                        pv,
                        xt_sb[:, kd, t * 128 : (t + 1) * 128],
                        wv_sb[:, kd, :],
                        start=(kd == 0),
                        stop=False,
                    )
                if half == 1:
                    state.pop(t)
                    nc.tensor.matmul(pv, ones128, bv_sb, start=False, stop=True)
                    nc.vector.tensor_copy(
                        v_sb[:, t, :, 0:DH],
                        pv.rearrange("p (h x) -> p h x", x=DH),
                    )

            def v_proj(pool, t):
                v_sub(pool, t, 0)
                v_sub(pool, t, 1)

            # ---- phase 1 (startup): the minimum for step (th0,h0,kt0) ----
            # m=1 chunks (head pair 2/3) are only needed from step 32, V and
            # the remaining K/Q chunks drain into phase-2 PE slack (a fill
            # feeding step i MUST be emitted at a fill slot <= i: the tile
            # scheduler keeps per-engine emission order).
            with tc.tile_pool(name="ps1", bufs=4, space="PSUM") as ps1:
                # PE p-state warmup on zeros during the DMA lead (the tensor
                # engine ramps 0.65->2.4GHz over ~3us of continuous work),
                # and pre-load the Exp activation table off the critical path
                junk = sp.tile([128, 512], f16, tag="junk", name="junk")
                escr = sp.tile([1, 128], f16, tag="escr", name="escr")
                nc.vector.memset(junk, 0.0)
                nc.vector.memset(escr, 0.0)
                nc.scalar.activation(escr, escr, AF.Exp)
                wut = ps1.tile([128, 512], f32, tag="p", name="wut")
                for w in range(9):
                    nc.tensor.matmul(
                        wut, junk[:, 0:128], junk, start=(w == 0), stop=(w == 8)
                    )
                qk_chunk(ps1, wk_sb, bk_sb, kt_sb, 0, 0)
                qk_chunk(ps1, wq_sb, bq_sb, qt_sb, 0, 0)
                qk_chunk(ps1, wq_sb, bq_sb, qt_sb, 1, 0)

            # ---- phase 2 ----
            steps = [
                (th, h, kt)
                for th in range(2)
                for h in range(NHC)
                for kt in range(KT)
            ]
            nsteps = len(steps)

            with (
                tc.tile_pool(name="ps_s", bufs=2, space="PSUM") as pss,
                tc.tile_pool(name="ps_u", bufs=1, space="PSUM") as psu,
            ):
                def s_matmuls(th, h, kt):
                    m, p0 = h // 2, (h % 2) * 64
                    with tc.high_priority(offset=24):
                        st = pss.tile([128, 1024], f32, tag="s")
                        for sub in range(2):
                            q0 = th * 1024 + sub * 512
                            nc.tensor.matmul(
                                st[:, sub * 512 : (sub + 1) * 512],
                                kt_sb[p0 : p0 + 64, m, kt * 128 : (kt + 1) * 128],
                                qt_sb[p0 : p0 + 64, m, q0 : q0 + 512],
                                start=True,
                                stop=True,
                            )
                    return st

                norm_tiles = {}
                u = None

                p3_state = {}

                def phase3_half(pool, t, n, copy_engine=None):
                    if n == 0:
                        p3_state[t] = op_.tile([128, 1024], f16, name="ot")
                    ot = p3_state[t]
                    po = pool.tile([128, 512], f32, tag="o")
                    for m in range(C // 128):
                        nc.tensor.matmul(
                            po,
                            hc_sb[:, m, t * 128 : (t + 1) * 128],
                            wo_sb[:, m, n * 512 : (n + 1) * 512],
                            start=(m == 0),
                            stop=(m == C // 128 - 1),
                        )
                    eng = copy_engine or nc.vector
                    if eng is nc.scalar:
                        nc.scalar.activation(
                            ot[:, n * 512 : (n + 1) * 512], po, AF.Identity
                        )
                    else:
                        eng.tensor_copy(ot[:, n * 512 : (n + 1) * 512], po)
                    if n == 1:
                        p3_state.pop(t)
                        nc.sync.dma_start(
                            out=out[t * 128 : (t + 1) * 128, :], in_=ot
                        )

                def transpose_quad(pst, m, th, lo):
                    # PE xbar-free transpose of 4 [128q,128c] norm tiles into
                    # hc layout; Pool copies them out of PSUM.
                    nt = norm_tiles[m]
                    tp = pst.tile([128, 8, 128], f16, tag="t", name="tp")
                    for j in range(4):
                        qt = th * 8 + lo + j
                        nc.tensor.matmul(
                            tp[:, lo + j, :],
                            nt[:, qt, :, :],
                            id_sb,
                            is_transpose=True,
                        )
                        nc.vector.tensor_copy(
                            hc_sb[:, m, qt * 128 : (qt + 1) * 128],
                            tp[:, lo + j, :],
                        )

                def do_step(i, fill_fn):
                    nonlocal u
                    th, h, kt = steps[i]
                    m = h // 2
                    if kt == 0:
                        # [q, 8*64 U | pad | 8 denom]  (2 PSUM banks)
                        u = psu.tile([128, 528], f32, tag="u")
                    st_cur = do_step.st_next
                    if i + 1 < nsteps:
                        do_step.st_next = s_matmuls(*steps[i + 1])
                    if fill_fn is not None:
                        fill_fn()
                    e = ep.tile([128, 1024], f16)
                    nc.scalar.activation(e, st_cur, AF.Exp)
                    nc.vector.tensor_mul(
                        e, e, mk_sb[:, kt, th * 1024 : (th + 1) * 1024]
                    )
                    # PSUM zero-regions are 2KB banks: only the first chain
                    # in each bank may set start=True (it pending-zeroes the
                    # whole bank); sibling chains at kt=0 then overwrite via
                    # the pending-zero flags.
                    for qtl in range(8):
                        nc.tensor.matmul(
                            u[:, qtl * DH : (qtl + 1) * DH],
                            e[:, qtl * 128 : (qtl + 1) * 128],
                            v_sb[:, kt, h, 0:DH],
                            start=(kt == 0 and qtl == 0),
                            stop=(kt == KT - 1),
                            skip_group_check=True,
                        )
                        nc.tensor.matmul(
                            u[:, 512 + qtl : 513 + qtl],
                            e[:, qtl * 128 : (qtl + 1) * 128],
                            v_sb[:, kt, h, DH : DH + 1],
                            start=(kt == 0 and qtl == 0),
                            stop=(kt == KT - 1),
                            skip_group_check=True,
                        )
                    if kt == KT - 1:
                        with tc.high_priority(offset=40):
                            if m not in norm_tiles:
                                norm_tiles[m] = npool.tile(
                                    [128, QT, 2, DH], f16, tag="n", name=f"nt{m}"
                                )
                            nt = norm_tiles[m]
                            rec = sp.tile([128, 8, 1], f32, tag="r")
                            nc.vector.reciprocal(
                                rec[:, :, 0], u[:, 512:520]
                            )
                            # single broadcast multiply (recip varies along
                            # the qt free dim -> 0-stride broadcast over dh)
                            nc.vector.tensor_tensor(
                                nt[:, th * 8 : (th + 1) * 8, h % 2, :],
                                u[:, 0:512].rearrange("p (a b) -> p a b", b=DH),
                                rec.to_broadcast((128, 8, DH)),
                                mybir.AluOpType.mult,
                            )

                do_step.st_next = s_matmuls(*steps[0])

                # --- T-half 0: drain remaining projection chunks ---
                # fill order is deadline-driven: V tile t is needed at step
                # t, K qc2/qc3 at steps 8/12, Q qc2-3 only at step 64.
                with tc.tile_pool(name="ps1b", bufs=1, space="PSUM") as ps1b:
                    # V tiles 0-3 off the exp(0) critical path (PE order:
                    # after S(0), before step 0's U matmuls)
                    for t in range(4):
                        v_proj(ps1b, t)

                    def K(qc, m):
                        return lambda: qk_chunk(ps1b, wk_sb, bk_sb, kt_sb, qc, m)

                    def Q(qc, m):
                        return lambda: qk_chunk(ps1b, wq_sb, bq_sb, qt_sb, qc, m)

                    def V(t):
                        return lambda: v_proj(ps1b, t)

                    # per-step fill schedule. deadlines (fill slot i runs
                    # after the S(i+1) prefetch): K qc_j m0 by slot 4j-2,
                    # V_t by slot t, m=1 chunks by ~slot 30+4j-2, Q qc2/3
                    # m0 by slot 62, m1 by slot 94.
                    def KS(qc, m, h):
                        return lambda: qk_sub(ps1b, wk_sb, bk_sb, kt_sb, qc, m, h)

                    def QS(qc, m, h):
                        return lambda: qk_sub(ps1b, wq_sb, bq_sb, qt_sb, qc, m, h)

                    def VS(t, h):
                        return lambda: v_sub(ps1b, t, h)

                    fill_sched = {
                        0: [VS(4, 0), KS(1, 0, 0)],
                        1: [VS(4, 1), KS(1, 0, 1)],
                        2: [VS(5, 0), VS(5, 1)],
                        3: [VS(6, 0), KS(2, 0, 0)],
                        4: [VS(6, 1), VS(7, 0)],
                        5: [VS(7, 1), KS(2, 0, 1)],
                        6: [VS(8, 0), VS(8, 1)],
                        7: [VS(9, 0), KS(3, 0, 0)],
                        8: [VS(9, 1), VS(10, 0)],
                        9: [VS(10, 1), KS(3, 0, 1)],
                        10: [VS(11, 0), VS(11, 1)],
                        11: [VS(12, 0)],
                        12: [VS(12, 1), VS(13, 0)],
                        13: [VS(13, 1)],
                        14: [VS(14, 0), VS(14, 1)],
                        15: [VS(15, 0), VS(15, 1)],
                        16: [KS(0, 1, 0)],
                        18: [KS(0, 1, 1)],
                        20: [QS(0, 1, 0)],
                        22: [QS(0, 1, 1)],
                        24: [QS(1, 1, 0)],
                        26: [QS(1, 1, 1)],
                        28: [KS(1, 1, 0)],
                        30: [KS(1, 1, 1)],
                        32: [KS(2, 1, 0)],
                        34: [KS(2, 1, 1)],
                        36: [KS(3, 1, 0)],
                        38: [KS(3, 1, 1)],
                        40: [QS(2, 0, 0)],
                        42: [QS(2, 0, 1)],
                        44: [QS(3, 0, 0)],
                        46: [QS(3, 0, 1)],
                        48: [QS(2, 1, 0)],
                        50: [QS(2, 1, 1)],
                        52: [QS(3, 1, 0)],
                        54: [QS(3, 1, 1)],
                    }
                    for i in range(nsteps // 2):
                        fs = fill_sched.get(i)
                        do_step(
                            i,
                            (lambda fs=fs: [f() for f in fs]) if fs else None,
                        )

                # --- T-half 1: transpose T-half-0 norms, then interleave
                # phase-3 t-tiles 0-7 into PE slack ---
                with (
                    tc.tile_pool(name="ps_t", bufs=1, space="PSUM") as pst,
                    tc.tile_pool(name="ps_o", bufs=1, space="PSUM") as pso,
                ):
                    fills2 = []
                    for m in range(2):
                        for lo in (0, 4):
                            fills2.append(
                                lambda m=m, lo=lo: transpose_quad(pst, m, 0, lo)
                            )
                    for t in range(8):
                        for n in range(2):
                            fills2.append(lambda t=t, n=n: phase3_half(pso, t, n))
                    fi = 0
                    for i in range(nsteps // 2, nsteps):
                        th, h, kt = steps[i]
                        fill = None
                        if fi < 4 or (fi < len(fills2) and (i - 64) % 3 == 2):
                            fill = fills2[fi]
                            fi += 1
                        do_step(i, fill)
                        if kt == KT - 1 and h % 2 == 1 and h < NHC - 1:
                            # pair-0 T-half-1 norms ready: transpose them
                            transpose_quad(pst, h // 2, 1, 0)
                            transpose_quad(pst, h // 2, 1, 4)
                    for f in fills2[fi:]:
                        f()
                    # last pair (m=1, th=1) transposes on the critical tail
                    with tc.high_priority(offset=40):
                        transpose_quad(pst, 1, 1, 0)
                        transpose_quad(pst, 1, 1, 4)

            # ---- phase 3 tail: t-tiles 8-15 (phase-2 pools closed) ----
            with tc.tile_pool(name="ps_ot", bufs=6, space="PSUM") as psot:
                for t in range(8, KT):
                    for n in range(2):
                        phase3_half(
                            psot, t, n,
                            copy_engine=nc.scalar if (t + n) % 2 else nc.vector,
                        )
    nc.compile()
    return nc


def _get_nc(repeat=1):
    key = ("nc", repeat)
    if key not in _CACHE:
        _CACHE[key] = _build(repeat)
    return _CACHE[key]


def _prep_core_inputs(c, x, mask, Wq, bq, Wk, bk, Wv, bv, Wo):
    b, hb = divmod(c, NCORES // B)
    sl = slice(hb * C, (hb + 1) * C)
    return {
        "xt": np.ascontiguousarray(x[b].T).astype(np.float16),
        "wq": np.ascontiguousarray(Wq[:, sl] * SCALE).astype(np.float16),
        "wk": np.ascontiguousarray(Wk[:, sl]).astype(np.float16),
        "wv": np.ascontiguousarray(Wv[:, sl]).astype(np.float16),
        "wo": np.ascontiguousarray(Wo[sl, :]).astype(np.float16),
        "maskt": np.ascontiguousarray(mask[b].T).astype(np.float16),
        "bqs": (bq[sl] * SCALE).astype(np.float32),
        "bks": bk[sl].astype(np.float32),
        "bvr": bv[sl].astype(np.float16).reshape(1, C),
        "ident": np.eye(128, dtype=np.float16),
    }


def kernel(
    inputs, mask, Wq, bq, Wk, bk, Wv, bv, Wo, bo,
    _trace=False, _trace_kwargs=None, _repeat=1,
):
    x = np.asarray(inputs, dtype=np.float32)
    mask = np.asarray(mask)
    Wq, bq = np.asarray(Wq, np.float32), np.asarray(bq, np.float32)
    Wk, bk = np.asarray(Wk, np.float32), np.asarray(bk, np.float32)
    Wv, bv = np.asarray(Wv, np.float32), np.asarray(bv, np.float32)
    Wo, bo = np.asarray(Wo, np.float32), np.asarray(bo, np.float32)

    nc = _get_nc(_repeat)
    in_maps = [
        _prep_core_inputs(c, x, mask, Wq, bq, Wk, bk, Wv, bv, Wo)
        for c in range(NCORES)
    ]
    last_err = None
    for attempt in range(3):
        try:
            res = run_bass_kernel_spmd(
                nc,
                in_maps,
                list(range(NCORES)),
                trace=_trace,
                **(_trace_kwargs or {}),
            )
            break
        except Exception as e:  # wedged device etc. -- retry
            last_err = e
            time.sleep(3.0)
    else:
        raise last_err
    out = np.empty((B, T, D), np.float32)
    per_b = NCORES // B
    for b in range(B):
        acc = res.results[b * per_b]["out"].astype(np.float32)
        for j in range(1, per_b):
            acc = acc + res.results[b * per_b + j]["out"].astype(np.float32)
        out[b] = acc + bo[None, :]
    if _trace:
        kernel.last_results = res
    return out
